# revision 1
# baseline (speedup 1.0000x reference)
"""Trainium2 Bass kernel for NeighborsValuesAssigner (retrieval_knn) — v2.

out[b,:,h,w] = mean_{n in top8} values[n]  where top8 = 8 smallest
dist[b,n,h,w] = 0.5||p_n||^2 - <p_n, x_patch(b,h,w)>  (5x5 'same' conv).

8 cores, data-parallel over batch (4 images/core). Per core, per group of
512 pixels:

  pass A  S_A[px,n] on PE as 2 K-packed accumulating fp16 matmuls
          (hi/lo split products xh*ph / xl*ph / xh*pl = 227 K-rows packed
          into K=128 + K=101; error ~2^-22).
  top8    DVE max8 per 512-chunk of n + merge -> t8 (8th largest score).
  t-rows  t8 split hi/lo fp16, PE-transposed to [2, 512px], written into
          partitions 96:98 of the group's X tile; the patch-side rows
          there are -1, so pass B computes S - t8 directly.  In pass A
          those X rows are host-zeroed, contributing exactly 0.
  pass B  S_B[n,px] - t8[px] on PE: same 2 K-packed matmuls.  S_B is
          bit-identical to S_A up to ~3 fp32 rounds (<1e-5), so the
          compare S_B - t8 >= -DELTA reproduces top-8 exactly except
          where the 8|9 score gap < DELTA (~60 px of 131072).
  mask    ACT sign(S_B - t8 + DELTA) -> {+-1} fp16 [n, px]  (no PE
          transposes needed: already n-major). Optionally some chunks on
          DVE as is_ge -> {0,1} with a 2x-scaled values table.
  value   PE out'[D,px] = sum_n mask[n,px]*values[n,D] over 16 chunks.
  final   ACT out = (out' + sum_n values[n])/16  (corrects the +-1
          convention; = mean of selected 8) -> DMA to DRAM.
"""
import sys

sys.path.insert(0, "/opt/trn_rl_repo")

import numpy as np

B, C, H, W = 32, 3, 64, 64
N, D = 2048, 128
KH = KW = 5
KDIM = C * KH * KW + 1      # 76 = 75 patch dims + bias row
NCORES = 8
BLOC = B // NCORES          # 4 images per core
PX = BLOC * H * W           # 16384 pixels per core
GPX = 512                   # pixels per group
NGRP = PX // GPX            # 32 groups per core
NCHUNK = N // 128           # 16 patch chunks (pass B / value)

# K-packing of the 3 hi/lo product sets (xh*ph 76, xl*ph 75, xh*pl 76):
# mm1: K=128 = (xh,ph)[0:76] + (xl,ph)[0:52]
# mm2: K=101 = (xl,ph)[52:75] + (xh,pl)[0:41] + 2 t-rows + (xh,pl)[41:76]
#       (t-rows at partitions TRI:TRI+2 = 64:66 so the PE transpose may
#        target them directly: bass restricts base partitions to {0,32,64})
KA = 128
TRI = 64
KBT = 101

DELTA = 1.5e-4              # threshold margin (abs score units)
DVE_CHUNKS = frozenset(range(0, 16, 2))  # even chunks: DVE is_ge {0,1} w/ 2x values; odd: ACT sign {+-1}

_CACHE = {}


def _build_program(loop_r=0):
    """loop_r=0: straight-line. loop_r>0: wrap body in a device-side
    For_i loop running it loop_r times (for HW timing via wall deltas)."""
    import concourse.bacc as bacc
    import concourse.tile as tile
    import concourse.mybir as mybir
    from contextlib import ExitStack

    f32 = mybir.dt.float32
    f16 = mybir.dt.float16
    nc = bacc.Bacc("TRN2", target_bir_lowering=False, debug=False)

    xa_d = nc.dram_tensor("xa", [KA, PX], f16, kind="ExternalInput").ap()
    xb_d = nc.dram_tensor("xb", [KBT, PX], f16, kind="ExternalInput").ap()
    pa_d = nc.dram_tensor("pa", [KA, N], f16, kind="ExternalInput").ap()
    pb_d = nc.dram_tensor("pb", [KBT, N], f16, kind="ExternalInput").ap()
    vs_d = nc.dram_tensor("vs16", [128, N], f16, kind="ExternalInput").ap()
    id_d = nc.dram_tensor("id16", [128, 128], f16, kind="ExternalInput").ap()
    sv_d = nc.dram_tensor("sumv16", [128, 1], f32, kind="ExternalInput").ap()
    dl_d = nc.dram_tensor("delta32", [128, 1], f32, kind="ExternalInput").ap()
    out = nc.dram_tensor("out", [BLOC, 128, H * W], f32, kind="ExternalOutput").ap()

    with tile.TileContext(nc) as tc, ExitStack() as ctx:
        const = ctx.enter_context(tc.tile_pool(name="const", bufs=1))
        xap = ctx.enter_context(tc.tile_pool(name="xap", bufs=3))
        xbp = ctx.enter_context(tc.tile_pool(name="xbp", bufs=3))
        mhp = ctx.enter_context(tc.tile_pool(name="mhp", bufs=8))
        mgp = ctx.enter_context(tc.tile_pool(name="mgp", bufs=2))
        ttp = ctx.enter_context(tc.tile_pool(name="ttp", bufs=2))
        mkp = ctx.enter_context(tc.tile_pool(name="mkp", bufs=4))
        otp = ctx.enter_context(tc.tile_pool(name="otp", bufs=2))
        psA = ctx.enter_context(tc.tile_pool(name="psA", bufs=3, space="PSUM"))
        psB = ctx.enter_context(tc.tile_pool(name="psB", bufs=3, space="PSUM"))
        psT = ctx.enter_context(tc.tile_pool(name="psT", bufs=1, space="PSUM"))
        psV = ctx.enter_context(tc.tile_pool(name="psV", bufs=1, space="PSUM"))

        pa_t = const.tile([KA, N], f16)
        pb_t = const.tile([KBT, N], f16)
        vs_t = const.tile([128, N], f16)
        id_t = const.tile([128, 128], f16)
        sv_t = const.tile([128, 1], f32)
        dl_t = const.tile([128, 1], f32)
        nc.sync.dma_start(pa_t[:], pa_d[:])
        nc.sync.dma_start(pb_t[:], pb_d[:])
        nc.sync.dma_start(vs_t[:], vs_d[:])
        nc.sync.dma_start(id_t[:], id_d[:])
        nc.sync.dma_start(sv_t[:], sv_d[:])
        nc.sync.dma_start(dl_t[:], dl_d[:])

        loop_cm = tc.For_i(0, loop_r, 1) if loop_r else None
        if loop_cm is not None:
            loop_cm.__enter__()

        grp_per_img = (H * W) // GPX  # 8
        for g in range(NGRP):
            b, s = divmod(g, grp_per_img)
            gsl = slice(g * GPX, (g + 1) * GPX)
            xa_t = xap.tile([KA, GPX], f16, tag="xa")
            xb_t = xbp.tile([KBT, GPX], f16, tag="xb")
            nc.sync.dma_start(xa_t[:], xa_d[:, gsl])
            nc.sync.dma_start(xb_t[:], xb_d[:, gsl])

            # ---- pass A: scores [px, n] + top-8 threshold ----
            mrg = mgp.tile([128, 32], f32, tag="mrg")  # 4 merged max8s
            for t in range(4):
                lq = xa_t[:, t * 128:(t + 1) * 128]
                lb = xb_t[:, t * 128:(t + 1) * 128]
                mh = mhp.tile([128, 32], f32, tag="mh")
                for c in range(4):  # n chunks of 512
                    nsl = slice(c * 512, (c + 1) * 512)
                    pA = psA.tile([128, 512], f32, tag="pA")
                    nc.tensor.matmul(pA[:], lq, pa_t[:, nsl],
                                     start=True, stop=False)
                    nc.tensor.matmul(pA[:], lb, pb_t[:, nsl],
                                     start=False, stop=True)
                    nc.vector.max(mh[:, c * 8:(c + 1) * 8], pA[:])
                nc.vector.max(mrg[:, t * 8:(t + 1) * 8], mh[:])

            # ---- t8 -> hi/lo f16 rows, transposed into xb_t[96:98] ----
            tt = ttp.tile([128, 8], f16, tag="tt")   # cols 2t=th, 2t+1=tl
            th32 = ttp.tile([128, 4], f32, tag="th32")
            t8s = mrg[:, 7::8]                       # [128, 4] strided
            nc.vector.tensor_copy(tt[:, 0::2], t8s)             # th = f16(t8)
            nc.vector.tensor_copy(th32[:], tt[:, 0::2])         # f32(th)
            nc.vector.tensor_sub(tt[:, 1::2], t8s, th32[:])     # tl = f16(t8-th)
            pT = psT.tile([128, GPX], f16, tag="pT")
            for t in range(4):
                nc.tensor.transpose(pT[TRI:TRI + 2, t * 128:(t + 1) * 128],
                                    tt[:, 2 * t:2 * t + 2], id_t[:])
            nc.scalar.copy(xb_t[TRI:TRI + 2, :], pT[TRI:TRI + 2, :])

            # ---- pass B: S - t8 in [n, px]; mask; value matmul ----
            pV = psV.tile([128, GPX], f32, tag="pV")
            for c in range(NCHUNK):
                csl = slice(c * 128, (c + 1) * 128)
                pB = psB.tile([128, GPX], f32, tag="pB")
                nc.tensor.matmul(pB[:], pa_t[:, csl], xa_t[:],
                                 start=True, stop=False)
                nc.tensor.matmul(pB[:], pb_t[:, csl], xb_t[:],
                                 start=False, stop=True)
                mk = mkp.tile([128, GPX], f16, tag="mk")
                if c in DVE_CHUNKS:  # {0,1} mask (values table 2x for these)
                    nc.vector.tensor_scalar(mk[:], pB[:], -DELTA, None,
                                            mybir.AluOpType.is_ge)
                else:                # {+-1} mask
                    nc.scalar.sign(mk[:], pB[:], bias=dl_t[:, 0:1])
                nc.tensor.matmul(pV[:], vs_t[:, csl], mk[:],
                                 start=(c == 0), stop=(c == NCHUNK - 1))

            ot = otp.tile([128, GPX], f32, tag="ot")
            nc.scalar.activation(ot[:], pV[:],
                                 mybir.ActivationFunctionType.Identity,
                                 bias=sv_t[:, 0:1], scale=1.0 / 16.0)
            nc.sync.dma_start(out[b, :, s * GPX:(s + 1) * GPX], ot[:])

        if loop_cm is not None:
            loop_cm.__exit__(None, None, None)

    nc.compile()
    return nc


def _get_program():
    if "nc" not in _CACHE:
        _CACHE["nc"] = _build_program()
    return _CACHE["nc"]


def _im2col(x):
    """x: (B,3,64,64) f32 -> cols (B, 75, 4096) f32, k=(c,dy,dx), px=(h,w)."""
    xpad = np.pad(x, ((0, 0), (0, 0), (2, 2), (2, 2)))
    win = np.lib.stride_tricks.sliding_window_view(xpad, (KH, KW), axis=(2, 3))
    cols = np.ascontiguousarray(win.transpose(0, 1, 4, 5, 2, 3))
    return cols.reshape(x.shape[0], KDIM - 1, H * W)


def _pack_b(xl_or_ph52, xh_or_pl, fill):
    """rows: [a(23); b[0:TRI-23]; fill(2); b[TRI-23:76]] -> [101, cols]."""
    a, bm = xl_or_ph52, xh_or_pl
    ncols = a.shape[1]
    out = np.empty((KBT, ncols), np.float16)
    out[0:23] = a
    out[23:TRI] = bm[0:TRI - 23]
    out[TRI:TRI + 2] = fill
    out[TRI + 2:KBT] = bm[TRI - 23:76]
    return out


def _host_prep(x, patches, values):
    """Returns per-core in_maps list."""
    pf = patches.reshape(N, KDIM - 1)
    bias = (-0.5 * np.sum(pf.astype(np.float64) ** 2, axis=1)).astype(np.float32)

    pfull = np.zeros((KDIM, N), np.float32)
    pfull[0:KDIM - 1] = pf.T
    pfull[KDIM - 1] = bias
    ph = pfull.astype(np.float16)
    pl = (pfull - ph.astype(np.float32)).astype(np.float16)

    pa = np.ascontiguousarray(np.concatenate([ph, ph[0:52]], axis=0))  # [128, N]
    pb = _pack_b(ph[52:75], pl, -1.0)                                  # [101, N]

    v16 = values.astype(np.float16)                                    # [N, D]
    scale = np.ones(N, np.float16)
    for c in DVE_CHUNKS:
        scale[c * 128:(c + 1) * 128] = 2.0
    vscaled = (v16 * scale[:, None]).astype(np.float16)
    vs16 = np.ascontiguousarray(
        vscaled.reshape(NCHUNK, 128, 128).transpose(1, 0, 2).reshape(128, N))
    act_n = [n for n in range(N) if (n // 128) not in DVE_CHUNKS]
    sumv = (np.sum(v16[act_n].astype(np.float64), axis=0)
            if act_n else np.zeros(D))
    sumv16 = (sumv / 16.0).astype(np.float32).reshape(D, 1)

    id16 = np.eye(128, dtype=np.float16)
    delta32 = np.full((128, 1), DELTA, np.float32)

    cols = _im2col(x)  # (32, 75, 4096) f32
    in_maps = []
    for i in range(NCORES):
        xfull = np.empty((KDIM, PX), np.float32)
        xfull[0:KDIM - 1] = np.concatenate(
            [cols[i * BLOC + j] for j in range(BLOC)], axis=1)
        xfull[KDIM - 1] = 1.0
        xh = xfull.astype(np.float16)
        xl = (xfull - xh.astype(np.float32)).astype(np.float16)
        xa = np.ascontiguousarray(np.concatenate([xh, xl[0:52]], axis=0))
        xb = _pack_b(xl[52:75], xh, 0.0)
        in_maps.append({"xa": xa, "xb": xb, "pa": pa, "pb": pb,
                        "vs16": vs16, "id16": id16, "sumv16": sumv16,
                        "delta32": delta32})
    return in_maps


def kernel(x, patches, values):
    from concourse.bass_utils import run_bass_kernel_spmd

    x = np.asarray(x, dtype=np.float32)
    patches = np.asarray(patches, dtype=np.float32)
    values = np.asarray(values, dtype=np.float32)

    nc = _get_program()
    in_maps = _host_prep(x, patches, values)
    res = run_bass_kernel_spmd(nc, in_maps, list(range(NCORES)))

    out = np.empty((B, D, H, W), np.float32)
    for i in range(NCORES):
        o = res.results[i]["out"]  # (BLOC, 128, 4096)
        out[i * BLOC:(i + 1) * BLOC] = o.reshape(BLOC, D, H, W)
    return out

